# revision 5
# baseline (speedup 1.0000x reference)
"""3-layer GraphSAGE (max-pool aggregator) on 8 trn2 NeuronCores.

Strategy (hardcoded for N=50000, E=800000, D=128, H=256, P=512, O=64):
  - Relabel nodes by in-degree (desc), block-interleaved across 8 cores so
    every core's i-th node tile has ~equal max degree (load balance + tight
    per-tile padding).
  - Each core owns 49 real tiles of 128 dst nodes. Per layer:
      h-shard = x_local @ aggW  (fp16, PE)  -> DRAM shard
      AllGather shards -> full h table (two halves A/B, each <32768 rows so
      int16 gather indices work)
      dma_gather (custom SWDGE transpose gather) pulls each tile's padded
      neighbor rows feature-major; DVE tree tensor_max reduces k-slices ->
      hN^T tile; PE computes concat([x, hN]) @ linW (+ReLU) directly from
      the feature-major tiles.
  - Sentinel table rows (-60000 / 0.0) implement segment-max padding and
    DGL's zero-fill for isolated nodes exactly.
"""

import sys

if "/opt/trn_rl_repo" not in sys.path:
    sys.path.insert(0, "/opt/trn_rl_repo")

import numpy as np

# ---- problem dims (hardcoded per spec) ----
N_NODES = 50000
DIM_D = 128
DIM_H = 256
DIM_P = 512
DIM_O = 64

NCORES = 8
NT = 128          # nodes per tile
CH = 16           # gather chunk (k-slices per dma_gather call)
NBIG = -60000.0   # -inf stand-in (fp16-safe)


def _geom(n_nodes):
    rt = -(-n_nodes // (NT * NCORES))          # real tiles per core
    pt = rt + 3
    if pt % 2:
        pt += 1                                # physical tiles per core (even)
    lt = pt // 2                               # tiles per half
    low_real = lt - 1                          # real tiles in low half
    high_real = rt - low_real
    assert 0 < high_real <= lt - 2, (rt, pt, lt)
    shard = pt * NT
    half = shard // 2
    assert NCORES * half < 32768, "half-table must be int16-indexable"
    return rt, pt, lt, shard, half


def _prep_graph(src, dst, n_nodes):
    """Host-side: relabel nodes, build per-core padded gather indices."""
    rt, pt, lt, shard, half = _geom(n_nodes)
    src = np.asarray(src).astype(np.int64)
    dst = np.asarray(dst).astype(np.int64)

    deg = np.bincount(dst, minlength=n_nodes)
    order = np.argsort(-deg, kind="stable")    # rank -> orig node
    ranks = np.arange(n_nodes)
    blk = ranks // NT
    pos = ranks % NT
    core = blk % NCORES
    rtile = blk // NCORES
    assert rtile.max() < rt
    ptile = np.where(rtile < lt - 1, rtile, rtile + 1)
    newid_of_rank = core * shard + ptile * NT + pos
    newid = np.empty(n_nodes, np.int64)
    newid[order] = newid_of_rank

    src_n = newid[src]
    dst_n = newid[dst]

    ec = dst_n // shard
    eloc = dst_n % shard
    ept = eloc // NT
    epos = eloc % NT
    er = np.where(ept < lt, ept, ept - 1)      # phys tile -> real tile idx
    # (real dst never sit in pad tiles; ept<lt-1 -> r=ept, ept>=lt -> r=ept-1)

    sc = src_n // shard
    sloc = src_n % shard
    isA = sloc < half
    arow = sc * half + sloc
    brow = sc * half + (sloc - half)

    cntA = np.zeros((NCORES, rt, NT), np.int32)
    cntB = np.zeros((NCORES, rt, NT), np.int32)
    np.add.at(cntA, (ec[isA], er[isA], epos[isA]), 1)
    np.add.at(cntB, (ec[~isA], er[~isA], epos[~isA]), 1)
    KA_site = np.maximum(cntA.max(axis=(0, 2)), 1)   # [rt]
    KB_site = np.maximum(cntB.max(axis=(0, 2)), 1)

    def k_within(mask):
        """k-rank of each edge within its (dst, half) group."""
        idx = np.flatnonzero(mask)
        o = idx[np.argsort(dst_n[idx], kind="stable")]
        d = dst_n[o]
        if len(d) == 0:
            return np.zeros(len(dst_n), np.int64)
        starts = np.r_[0, np.flatnonzero(np.diff(d)) + 1]
        lens = np.diff(np.r_[starts, len(d)])
        k_sorted = np.arange(len(d)) - np.repeat(starts, lens)
        karr = np.zeros(len(dst_n), np.int64)
        karr[o] = k_sorted
        return karr

    kA = k_within(isA)
    kB = k_within(~isA)

    offA = np.r_[0, np.cumsum(KA_site)] * NT   # flat idx offsets per site
    offB = np.r_[0, np.cumsum(KB_site)] * NT
    totalA = int(offA[-1])
    totalB = int(offB[-1])

    NROW_A = (lt - 1) * NT          # phys tile lt-1, pos 0 (low half pad)
    ZROW_A = NROW_A + 1
    NROW_B = (pt - 1 - lt) * NT     # phys tile pt-1, pos 0 (high half pad)
    ZROW_B = NROW_B + 1

    flatA = np.full((NCORES, totalA), NROW_A, np.int32)
    pA = offA[er[isA]] + kA[isA] * NT + epos[isA]
    flatA[ec[isA], pA] = arow[isA]
    flatB = np.full((NCORES, totalB), NROW_B, np.int32)
    pB = offB[er[~isA]] + kB[~isA] * NT + epos[~isA]
    flatB[ec[~isA], pB] = brow[~isA]

    z = np.flatnonzero(deg == 0)
    if len(z):
        zn = newid[z]
        zc = zn // shard
        zl = zn % shard
        zpt = zl // NT
        zr = np.where(zpt < lt, zpt, zpt - 1)
        zpos = zl % NT
        flatA[zc, offA[zr] + zpos] = ZROW_A

    assert flatA.max() < 32768 and flatB.max() < 32768

    def wrap(flat):
        t = flat.shape[1]
        a = flat.astype(np.int16).reshape(NCORES, t // 16, 16).transpose(0, 2, 1)
        return np.ascontiguousarray(np.tile(a, (1, 8, 1)))  # [NCORES,128,t//16]

    return dict(
        geom=(rt, pt, lt, shard, half),
        newid=newid,
        idxA=wrap(flatA),
        idxB=wrap(flatB),
        KA_site=KA_site.astype(int),
        KB_site=KB_site.astype(int),
        offA=offA.astype(int),
        offB=offB.astype(int),
    )


def _build_program(geom, KA_site, KB_site, offA, offB, totalA, totalB,
                   dim_d, dim_h, dim_p, dim_o):
    import concourse.bacc as bacc
    import concourse.mybir as mybir
    import concourse.tile as tile
    from concourse.library_config import mlp

    fp16 = mybir.dt.float16
    f32 = mybir.dt.float32
    i16 = mybir.dt.int16
    Relu = mybir.ActivationFunctionType.Relu

    rt, pt, lt, shard, half = geom
    PC = dim_p // 128
    layer_dims = [(dim_d, dim_h, True), (dim_h, dim_h, True), (dim_h, dim_o, False)]

    nc = bacc.Bacc(
        "TRN2",
        num_devices=NCORES,
        debug=False,
        target_bir_lowering=False,
        dynamic_dma_scratch_size=32768,
    )

    xt0_d = nc.dram_tensor("xt0", [128, shard], fp16, kind="ExternalInput")
    idxA_d = nc.dram_tensor("idxA", [128, totalA // 16], i16, kind="ExternalInput")
    idxB_d = nc.dram_tensor("idxB", [128, totalB // 16], i16, kind="ExternalInput")
    padrows_d = nc.dram_tensor("padrows", [128, dim_p], fp16, kind="ExternalInput")
    aggw_d = []
    linw_d = []
    for li, (din, dout, _) in enumerate(layer_dims):
        kd = din // 128
        aggw_d.append(
            nc.dram_tensor(f"aggw{li}", [128, kd, dim_p], fp16, kind="ExternalInput")
        )
        linw_d.append(
            nc.dram_tensor(f"linw{li}", [128, kd + PC, dout], fp16, kind="ExternalInput")
        )
    out_d = nc.dram_tensor("out", [shard, dim_o], f32, kind="ExternalOutput")

    def phys(r):
        return r if r < lt - 1 else r + 1

    with tile.TileContext(nc) as tc:
        with (
            tc.tile_pool(name="const", bufs=1) as const,
            tc.tile_pool(name="work", bufs=3) as work,
            tc.tile_pool(name="gp", bufs=2) as gp,
            tc.tile_pool(name="ps", bufs=2, space="PSUM") as ps,
            tc.tile_pool(name="dram", bufs=1, space="DRAM") as dram,
        ):
            nc.gpsimd.load_library(mlp)

            # persistent SBUF state
            xta = const.tile([128, 2, shard], fp16, tag="xta")
            xtb = const.tile([128, 2, shard], fp16, tag="xtb")
            idxA_sb = const.tile([128, totalA // 16], i16, tag="idxA_sb")
            idxB_sb = const.tile([128, totalB // 16], i16, tag="idxB_sb")
            padrows = const.tile([128, dim_p], fp16, tag="padrows")
            nc.sync.dma_start(xta[:, 0, :], xt0_d[:])
            nc.sync.dma_start(idxA_sb[:], idxA_d[:])
            nc.sync.dma_start(idxB_sb[:], idxB_d[:])
            nc.sync.dma_start(padrows[:], padrows_d[:])
            aggw_sb = []
            linw_sb = []
            for li, (din, dout, _) in enumerate(layer_dims):
                kd = din // 128
                aw = const.tile([128, kd, dim_p], fp16, tag=f"aggw{li}",
                                name=f"aggw{li}_sb")
                nc.sync.dma_start(aw[:], aggw_d[li][:])
                lw = const.tile([128, kd + PC, dout], fp16, tag=f"linw{li}",
                                name=f"linw{li}_sb")
                nc.sync.dma_start(lw[:], linw_d[li][:])
                aggw_sb.append(aw)
                linw_sb.append(lw)

            # DRAM shards/tables, double-buffered by layer parity
            shardA = [dram.tile([half, dim_p], fp16, tag=f"shardA{i}",
                                name=f"shardA{i}") for i in range(3)]
            shardB = [dram.tile([half, dim_p], fp16, tag=f"shardB{i}",
                                name=f"shardB{i}") for i in range(3)]
            tableA = [dram.tile([NCORES * half, dim_p], fp16, addr_space="Shared",
                                tag=f"tableA{i}", name=f"tableA{i}") for i in range(3)]
            tableB = [dram.tile([NCORES * half, dim_p], fp16, addr_space="Shared",
                                tag=f"tableB{i}", name=f"tableB{i}") for i in range(3)]

            for li, (din, dout, act) in enumerate(layer_dims):
                par = li
                kd = din // 128
                xin = xta if li % 2 == 0 else xtb
                xout = xtb if li % 2 == 0 else xta
                aggw = aggw_sb[li]
                linw = linw_sb[li]

                # ---- phase A: h shard = x_local @ aggW ----
                for r in range(rt):
                    p_ = phys(r)
                    ph = ps.tile([128, dim_p], f32, tag="ph", name="ph")
                    for t in range(kd):
                        nc.tensor.matmul(
                            ph[:],
                            xin[:, t, p_ * NT:(p_ + 1) * NT],
                            aggw[:, t, :],
                            start=(t == 0),
                            stop=(t == kd - 1),
                        )
                    h16 = work.tile([128, dim_p], fp16, tag="h16", name="h16")
                    nc.scalar.copy(h16[:], ph[:])
                    if p_ < lt:
                        dst_ap = shardA[par][p_ * NT:(p_ + 1) * NT, :]
                    else:
                        dst_ap = shardB[par][(p_ - lt) * NT:(p_ - lt + 1) * NT, :]
                    nc.sync.dma_start(dst_ap, h16[:])
                # pad tiles (sentinel rows)
                real_phys = {phys(r) for r in range(rt)}
                for p_ in range(pt):
                    if p_ in real_phys:
                        continue
                    if p_ < lt:
                        dst_ap = shardA[par][p_ * NT:(p_ + 1) * NT, :]
                    else:
                        dst_ap = shardB[par][(p_ - lt) * NT:(p_ - lt + 1) * NT, :]
                    nc.sync.dma_start(dst_ap, padrows[:])

                # ---- phase B: allgather halves ----
                nc.gpsimd.collective_compute(
                    "AllGather",
                    mybir.AluOpType.bypass,
                    replica_groups=[list(range(NCORES))],
                    ins=[shardA[par][:].opt()],
                    outs=[tableA[par][:].opt()],
                )
                nc.gpsimd.collective_compute(
                    "AllGather",
                    mybir.AluOpType.bypass,
                    replica_groups=[list(range(NCORES))],
                    ins=[shardB[par][:].opt()],
                    outs=[tableB[par][:].opt()],
                )

                # ---- phase C/D: gather -> max-reduce -> linear ----
                for r in range(rt):
                    p_ = phys(r)
                    hN = work.tile([128, PC, NT], fp16, tag="hN", name="hN")
                    first = True
                    for table, off, Ks, idx_sb in (
                        (tableA[par], offA, KA_site, idxA_sb),
                        (tableB[par], offB, KB_site, idxB_sb),
                    ):
                        K = int(Ks[r])
                        base = int(off[r])
                        k0 = 0
                        while k0 < K:
                            kc = min(CH, K - k0)
                            nidx = kc * NT
                            g = gp.tile([128, PC, nidx], fp16, tag="g", name="g")
                            c0 = (base + k0 * NT) // 16
                            nc.gpsimd.dma_gather(
                                g[:],
                                table[:],
                                idx_sb[:, c0:c0 + nidx // 16],
                                nidx,
                                nidx,
                                dim_p,
                                transpose=True,
                                single_packet=False,
                            )
                            kk = kc
                            while kk > 1:
                                hh = kk // 2
                                lo = kk - hh
                                nc.vector.tensor_max(
                                    g[:, :, 0:hh * NT],
                                    g[:, :, 0:hh * NT],
                                    g[:, :, lo * NT:kk * NT],
                                )
                                kk = lo
                            if first:
                                nc.vector.tensor_copy(hN[:], g[:, :, 0:NT])
                                first = False
                            else:
                                nc.vector.tensor_max(hN[:], hN[:], g[:, :, 0:NT])
                            k0 += kc

                    if act:
                        for hc in range(dout // 128):
                            po = ps.tile([128, NT], f32, tag="po", name="po")
                            for t in range(kd):
                                nc.tensor.matmul(
                                    po[:],
                                    linw[:, t, hc * 128:(hc + 1) * 128],
                                    xin[:, t, p_ * NT:(p_ + 1) * NT],
                                    start=(t == 0),
                                    stop=False,
                                )
                            for t4 in range(PC):
                                nc.tensor.matmul(
                                    po[:],
                                    linw[:, kd + t4, hc * 128:(hc + 1) * 128],
                                    hN[:, t4, :],
                                    start=False,
                                    stop=(t4 == PC - 1),
                                )
                            nc.scalar.activation(
                                xout[:, hc, p_ * NT:(p_ + 1) * NT], po[:], Relu
                            )
                    else:
                        po = ps.tile([128, dim_o], f32, tag="po", name="po")
                        for t in range(kd):
                            nc.tensor.matmul(
                                po[:],
                                xin[:, t, p_ * NT:(p_ + 1) * NT],
                                linw[:, t, 0:dim_o],
                                start=(t == 0),
                                stop=False,
                            )
                        for t4 in range(PC):
                            nc.tensor.matmul(
                                po[:],
                                hN[:, t4, :],
                                linw[:, kd + t4, 0:dim_o],
                                start=False,
                                stop=(t4 == PC - 1),
                            )
                        o32 = work.tile([128, dim_o], f32, tag="o32", name="o32")
                        nc.scalar.copy(o32[:], po[:])
                        nc.sync.dma_start(out_d[p_ * NT:(p_ + 1) * NT, :], o32[:])

    nc.compile()
    return nc


def _weights_maps(aggWs, linWs, dim_p):
    maps = {}
    for li, (aggW, linW) in enumerate(zip(aggWs, linWs)):
        din = aggW.shape[0]
        kd = din // 128
        maps[f"aggw{li}"] = np.ascontiguousarray(
            aggW.astype(np.float16).reshape(kd, 128, dim_p).transpose(1, 0, 2)
        )
        kt = linW.shape[0] // 128
        dout = linW.shape[1]
        maps[f"linw{li}"] = np.ascontiguousarray(
            linW.astype(np.float16).reshape(kt, 128, dout).transpose(1, 0, 2)
        )
    return maps


def _run(x, src, dst, aggWs, linWs, n_nodes, dim_d, dim_h, dim_p, dim_o):
    from concourse.bass_utils import run_bass_kernel_spmd

    prep = _prep_graph(src, dst, n_nodes)
    rt, pt, lt, shard, half = prep["geom"]
    totalA = prep["idxA"].shape[2] * 16
    totalB = prep["idxB"].shape[2] * 16

    nc = _build_program(
        prep["geom"], prep["KA_site"], prep["KB_site"], prep["offA"], prep["offB"],
        totalA, totalB, dim_d, dim_h, dim_p, dim_o,
    )

    newid = prep["newid"]
    X = np.zeros((NCORES * shard, dim_d), np.float16)
    X[newid] = np.asarray(x, np.float32).astype(np.float16)
    padrows = np.full((128, dim_p), NBIG, np.float16)
    padrows[1, :] = 0.0
    wmaps = _weights_maps(aggWs, linWs, dim_p)

    in_maps = []
    for c in range(NCORES):
        m = dict(wmaps)
        m["xt0"] = np.ascontiguousarray(X[c * shard:(c + 1) * shard].T)
        m["idxA"] = prep["idxA"][c]
        m["idxB"] = prep["idxB"][c]
        m["padrows"] = padrows
        in_maps.append(m)

    res = run_bass_kernel_spmd(nc, in_maps, core_ids=list(range(NCORES)))
    big = np.concatenate([res.results[c]["out"] for c in range(NCORES)], axis=0)
    return np.ascontiguousarray(big[newid]).astype(np.float32)


def kernel(x, src, dst, aggW0, aggW1, aggW2, linW0, linW1, linW2):
    return _run(
        np.asarray(x, np.float32),
        np.asarray(src),
        np.asarray(dst),
        [np.asarray(aggW0, np.float32), np.asarray(aggW1, np.float32),
         np.asarray(aggW2, np.float32)],
        [np.asarray(linW0, np.float32), np.asarray(linW1, np.float32),
         np.asarray(linW2, np.float32)],
        N_NODES, DIM_D, DIM_H, DIM_P, DIM_O,
    )
